# revision 22
# baseline (speedup 1.0000x reference)
"""Clustered Linformer Attention — Trainium2 Bass kernel, 8 NeuronCores.

Strategy: data-parallel over batch (2 batches/core, no collectives).
Math restructuring (verified vs reference to ~7e-7 in f32):
  - mask is all-ones => cluster c holds positions [32c, 32c+32); the per-head
    gather+einsum projections become  k_proj = AE[h]^T @ k_h  with a host-built
    sparse table AE[h] in [S, P] (score scale folded in), same for v with AF.
  - the 3-kernel conv fusion over scores collapses to 5 "tap" matrices M_t in
    [P, P] (t in -2..2):  scores_conv[s] = sum_t  (q[s+t] @ (k_proj^T @ M_t)).
    Taps are applied as 5 PSUM-accumulated matmuls with a column-shifted
    (zero-padded) q^T operand.
  - adjacent heads are packed block-diagonally so every matmul contracts over
    the full 128 partitions.
  - softmax has no max-subtraction (|scores| <~ 1.6, exp is safe in f32);
    Z = sum_c exp is computed by an all-ones block-diag matmul that also
    broadcasts Z to all 128 partitions, so normalization is one DVE op.

Perf notes (~203us, from 242us baseline; PE-busy ~100% of the body at
2.4GHz, ~173us matmul floor + ~17us fixed framework preamble/barrier):
  - all host tensors are pre-laid-out so every DMA moves contiguous >=2KB
    per-partition lines; ae/af are loaded once, resident in SBUF.
  - DMA queue ORDER = transfer priority: the sync/gpsimd queues lead with
    exactly what the first kv matmuls need (wk + leading x slices); batch-1
    x and the projection tables queue behind (consumed ~45us in).
  - ~115 dummy N=128 matmuls on a memset scratch tile burn the dead time
    before the first loads land (fixed ~7us preamble + ~9us DMA ramp), so
    the PE clock-gate (HAM) is warm for the entire kernel.
  - softmax is software-pipelined: the Z/attnV matmuls of round k-1 are
    emitted after the tap matmuls of round k, so the in-order PE queue never
    waits on the ACT exp latency.
  - dense bias matmuls are skipped when dense_b == 0 (it is); batch-1's last
    s-chunk runs dense as per-pair PSUM partials so the kernel tail is short.
"""
import sys
import numpy as np
import ml_dtypes

sys.path.insert(0, '/opt/trn_rl_repo')

B, S, D = 16, 2048, 512
H, P, C = 8, 64, 32
DEPTH = D // H           # 64
NCORES = 8
BLOC = B // NCORES       # 2 batches per core
NPAIR = H // 2           # 4 head pairs
SCH = 4                  # s-chunks of 512
SCW = S // SCH           # 512
NJ = S // 128            # 16 s-tiles of 128
NDC = D // 128           # 4 contraction chunks

_CACHE = {}


def _build_nc(use_bias):
    import concourse.tile as tile
    from concourse import mybir, bacc

    f32 = mybir.dt.float32
    bf16 = mybir.dt.bfloat16

    nc = bacc.Bacc()
    # all dram params pre-laid-out on host so DMAs are contiguous
    xT = nc.declare_dram_parameter("xT", [BLOC, D, S], bf16, isOutput=False)
    wq = nc.declare_dram_parameter("wq", [128, NDC * D], bf16, isOutput=False)
    wk = nc.declare_dram_parameter("wk", [128, NDC * D], bf16, isOutput=False)
    wv = nc.declare_dram_parameter("wv", [128, NDC * D], bf16, isOutput=False)
    dw = nc.declare_dram_parameter("dw", [128, NDC * D], bf16, isOutput=False)
    dbb = nc.declare_dram_parameter("dbb", [1, D], bf16, isOutput=False)
    ae = nc.declare_dram_parameter("ae", [NPAIR, 128, NJ * 128], bf16,
                                   isOutput=False)
    af = nc.declare_dram_parameter("af", [NPAIR, 128, NJ * 128], bf16,
                                   isOutput=False)
    bdm = nc.declare_dram_parameter("bdm", [128, 5 * 128], bf16, isOutput=False)
    onesbd = nc.declare_dram_parameter("onesbd", [128, 128], bf16,
                                       isOutput=False)
    out = nc.declare_dram_parameter("out", [BLOC, S, D], f32, isOutput=True)

    with tile.TileContext(nc) as tc:
        with tc.tile_pool(name="const", bufs=1) as cpool, \
             tc.tile_pool(name="big", bufs=1) as bigp, \
             tc.tile_pool(name="sm", bufs=3) as smp, \
             tc.tile_pool(name="bd", bufs=4) as bdp, \
             tc.tile_pool(name="ob", bufs=4) as obp, \
             tc.tile_pool(name="psB", bufs=4, space="PSUM") as psB:

            # ---- startup DMA priority. The framework preamble means no DMA
            # issues before ~7.2us, so queue ORDER is what matters: the two
            # usable queues (sync, gpsimd) lead with exactly what the first
            # kv matmuls need (wk + small leading x slices), and everything
            # non-critical (wq/dw, tables, batch-1 x) sits behind it.
            xt = [[bigp.tile([128, S], bf16, tag="xt", bufs=2 * NDC,
                             name=f"xt_{b}_{dc}") for dc in range(NDC)]
                  for b in range(BLOC)]
            wq_sb = cpool.tile([128, NDC, D], bf16)
            wk_sb = cpool.tile([128, NDC, D], bf16)
            wv_sb = cpool.tile([128, NDC, D], bf16)
            dw_sb = cpool.tile([128, NDC, D], bf16)
            HS = S // 2
            # warm-up scratch: memset on gpsimd (its preamble ends earliest)
            # before any DMA issues so the dummy matmuls can start ~7.3us.
            dummy = bigp.tile([128, 256], bf16, tag="warm")
            nc.gpsimd.memset(dummy, 0.0)
            # Batch-0 x arrives as S-quarters in the order the dc-outer
            # k-groups consume them, interleaved with the wk halves; the
            # first real matmul only needs wk[dc0,1] + x[dc0][0:512].
            nc.sync.dma_start(out=wk_sb[:, 0:2, :], in_=wk[:, 0:2 * D])
            nc.sync.dma_start(out=xt[0][0][:, 0:SCW],
                              in_=xT[0, 0:128, 0:SCW])
            nc.sync.dma_start(out=xt[0][1][:, 0:SCW],
                              in_=xT[0, 128:256, 0:SCW])
            nc.sync.dma_start(out=wk_sb[:, 2:4, :], in_=wk[:, 2 * D:4 * D])
            nc.sync.dma_start(out=xt[0][0][:, SCW:HS],
                              in_=xT[0, 0:128, SCW:HS])
            nc.sync.dma_start(out=xt[0][1][:, SCW:HS],
                              in_=xT[0, 128:256, SCW:HS])
            nc.sync.dma_start(out=wv_sb, in_=wv[:])
            nc.sync.dma_start(out=wq_sb, in_=wq[:])
            nc.sync.dma_start(out=dw_sb, in_=dw[:])
            # scalar + gpsimd queues: the other two x tiles' quarters in
            # parallel streams (early per-queue DMA bandwidth is the startup
            # bottleneck), then batch-0 second halves.
            nc.scalar.dma_start(out=xt[0][2][:, 0:SCW],
                                in_=xT[0, 256:384, 0:SCW])
            nc.scalar.dma_start(out=xt[0][3][:, 0:SCW],
                                in_=xT[0, 384:512, 0:SCW])
            nc.gpsimd.dma_start(out=xt[0][2][:, SCW:HS],
                                in_=xT[0, 256:384, SCW:HS])
            nc.gpsimd.dma_start(out=xt[0][3][:, SCW:HS],
                                in_=xT[0, 384:512, SCW:HS])
            for dc in range(NDC):
                nc.gpsimd.dma_start(
                    out=xt[0][dc][:, HS:S],
                    in_=xT[0, 128 * dc:128 * (dc + 1), HS:S])
            bdm_sb = cpool.tile([128, 5, 128], bf16)
            nc.sync.dma_start(out=bdm_sb, in_=bdm[:])
            ones_sb = cpool.tile([128, 128], bf16)
            nc.sync.dma_start(out=ones_sb, in_=onesbd[:])
            # dense bias (only when nonzero): applied as a k=1 accumulating
            # matmul (ones[1,128]^T @ bias_row[1,512]).
            bias_row = cpool.tile([1, D], bf16)
            nc.sync.dma_start(out=bias_row, in_=dbb[:])
            ones_col = cpool.tile([1, 128], bf16)
            nc.gpsimd.memset(ones_col, 1.0)
            ae_sb = cpool.tile([128, NPAIR, NJ, 128], bf16)
            af_sb = cpool.tile([128, NPAIR, NJ, 128], bf16)
            for pr in range(NPAIR):
                nc.sync.dma_start(out=ae_sb[:, pr, :, :], in_=ae[pr])
                nc.sync.dma_start(out=af_sb[:, pr, :, :], in_=af[pr])
            # batch-1 x at the very back of the sync queue: it must not
            # compete with the startup-critical loads, and still lands well
            # before its first consumer (~45us in).
            for dc in range(NDC):
                nc.sync.dma_start(out=xt[1][dc],
                                  in_=xT[1, 128 * dc:128 * (dc + 1), :])

            # Per-batch state; stages are emitted as closures so the two
            # batches can be interleaved in PE program order (engines execute
            # in order -- without interleaving, batch 1's QKV sits behind
            # batch 0's softmax gaps instead of filling them).
            st = [dict(expt={}) for _ in range(BLOC)]

            def emit_k(b, j, which=("knat", "vnat")):
                s = st[b]
                if j == 0 and "knat" in which:
                    s["knat"] = bigp.tile([128, NJ, D], bf16, tag="knat",
                                          name=f"knat_{b}")
                    s["vnat"] = bigp.tile([128, NJ, D], bf16, tag="vnat",
                                          name=f"vnat_{b}")
                for w_sb, key in ((wk_sb, "knat"), (wv_sb, "vnat")):
                    if key not in which:
                        continue
                    ps_k = psB.tile([128, D], f32, tag="ps512")
                    for dc in range(NDC):
                        nc.tensor.matmul(
                            ps_k,
                            xt[b][dc][:, 128 * j:128 * (j + 1)],
                            w_sb[:, dc, :],
                            start=(dc == 0), stop=(dc == NDC - 1))
                    if key == "knat":
                        nc.vector.tensor_copy(out=s[key][:, j, :], in_=ps_k)
                    else:
                        nc.scalar.copy(out=s[key][:, j, :], in_=ps_k)

            def emit_kv(b, j):
                emit_k(b, j)

            def emit_k_group(b, jg):
                # dc-outer k for 4 j-tiles at once: each dc pass only needs
                # wk[:,dc,:] + x[dc][jg quarter], so matmuls start as soon as
                # the first quarter lands (~9us) instead of after all of x.
                # Copies alternate DVE/ACT so the 4 held PSUM slots release
                # fast enough for the next group.
                s = st[b]
                ps = []
                for dc in range(NDC):
                    for jj in range(4):
                        j = 4 * jg + jj
                        if dc == 0:
                            ps.append(psB.tile([128, D], f32, tag="ps512",
                                               name=f"psk_{b}_{j}"))
                        nc.tensor.matmul(
                            ps[jj],
                            xt[b][dc][:, 128 * j:128 * (j + 1)],
                            wk_sb[:, dc, :],
                            start=(dc == 0), stop=(dc == NDC - 1))
                for jj in range(4):
                    j = 4 * jg + jj
                    if jj % 2 == 0:
                        nc.vector.tensor_copy(out=s["knat"][:, j, :],
                                              in_=ps[jj])
                    else:
                        nc.scalar.copy(out=s["knat"][:, j, :], in_=ps[jj])

            def emit_qt(b, pr, n):
                s = st[b]
                if pr == 0 and n == 0:
                    s["qt"] = bigp.tile([128, NPAIR, SCW * SCH + 4], bf16,
                                        tag="qT", bufs=2, name=f"qt_{b}")
                    nc.vector.memset(s["qt"][:, :, 0:2], 0.0)
                    nc.vector.memset(s["qt"][:, :, SCW * SCH + 2:], 0.0)
                ps_q = psB.tile([128, SCW], f32, tag="ps512")
                for dc in range(NDC):
                    nc.tensor.matmul(
                        ps_q,
                        wq_sb[:, dc, 128 * pr:128 * (pr + 1)],
                        xt[b][dc][:, SCW * n:SCW * (n + 1)],
                        start=(dc == 0), stop=(dc == NDC - 1))
                nc.scalar.copy(
                    out=s["qt"][:, pr, 2 + SCW * n:2 + SCW * (n + 1)],
                    in_=ps_q)

            def emit_proj(b, pr, psS):
                s = st[b]
                if pr == 0:
                    s["kp"] = bigp.tile([128, NPAIR, 128], bf16, tag="kpbd",
                                        bufs=2, name=f"kp_{b}")
                    s["vp"] = bigp.tile([128, NPAIR, 128], bf16, tag="vpbd",
                                        bufs=2, name=f"vp_{b}")
                    nc.vector.memset(s["kp"], 0.0)
                    nc.vector.memset(s["vp"], 0.0)
                for a_sb, key, dstk in ((ae_sb, "knat", "kp"),
                                        (af_sb, "vnat", "vp")):
                    # lhsT = [A_h0 | A_h1] columns, rhs = both heads' k/v
                    # columns; out diag blocks = the two k_proj's, off-diag
                    # blocks are cross-head garbage and are not copied.
                    ps_p = psS.tile([128, 128], f32, tag="pssmall")
                    for j in range(NJ):
                        nc.tensor.matmul(
                            ps_p,
                            a_sb[:, pr, j, :],
                            st[b][key][:, j, 128 * pr:128 * (pr + 1)],
                            start=(j == 0), stop=(j == NJ - 1))
                    dst = st[b][dstk]
                    nc.vector.tensor_copy(
                        out=dst[0:64, pr, 0:64], in_=ps_p[0:64, 0:64])
                    nc.vector.tensor_copy(
                        out=dst[64:128, pr, 64:128], in_=ps_p[64:128, 64:128])

            def emit_kt(b, pr, psS):
                s = st[b]
                if pr == 0:
                    s["concat"] = bigp.tile([128, NPAIR, S], bf16,
                                            tag="concatT", bufs=2,
                                            name=f"concat_{b}")
                    s["bdt"] = {}
                bdt = bdp.tile([128, 5, 128], bf16, tag="bdt",
                               name=f"bdt_{b}_{pr}")
                s["bdt"][pr] = bdt
                for t in range(5):
                    ps_b = psS.tile([128, 128], f32, tag="pssmall")
                    nc.tensor.matmul(ps_b, s["kp"][:, pr, :], bdm_sb[:, t, :],
                                     start=True, stop=True)
                    nc.vector.tensor_copy(out=bdt[:, t, :], in_=ps_b)

            def emit_att_a(b, pr, n):
                # 5 shifted tap matmuls -> scores, then ACT exp (async).
                s = st[b]
                bdt = s["bdt"][pr]
                ps_sc = psB.tile([128, SCW], f32, tag="ps512")
                for ti in range(5):  # t = ti - 2
                    nc.tensor.matmul(
                        ps_sc,
                        bdt[:, ti, :],
                        s["qt"][:, pr, SCW * n + ti:SCW * n + ti + SCW],
                        start=(ti == 0), stop=(ti == 4))
                expt = smp.tile([128, SCW], bf16, tag="expt")
                nc.scalar.activation(
                    out=expt, in_=ps_sc,
                    func=mybir.ActivationFunctionType.Exp)
                s["expt"][(pr, n)] = expt

            def emit_att_b(b, pr, n):
                # Z + attn@V matmuls (consume exp of an earlier round), then
                # DVE normalize into concat.
                s = st[b]
                expt = s["expt"].pop((pr, n))
                ps_z = psB.tile([128, SCW], f32, tag="ps512")
                nc.tensor.matmul(ps_z, ones_sb, expt, start=True, stop=True)
                ps_at = psB.tile([128, SCW], f32, tag="ps512")
                nc.tensor.matmul(ps_at, s["vp"][:, pr, :], expt,
                                 start=True, stop=True)
                # 1/Z: approx reciprocal (~18 bits, single DVE op). Exact
                # reciprocal is ~3.3us/tile; ACT ln/exp thrashes the table.
                rzb = smp.tile([128, SCW], f32, tag="rzb", bufs=2)
                nc.vector.reciprocal_approx_fast(out=rzb, in_=ps_z)
                nc.vector.tensor_mul(
                    out=s["concat"][:, pr, SCW * n:SCW * (n + 1)],
                    in0=ps_at, in1=rzb)

            def finish_dense(b, j, ps_d, veng):
                if use_bias:
                    nc.tensor.matmul(ps_d, ones_col, bias_row,
                                     start=False, stop=True)
                obuf = obp.tile([128, D], f32, tag="obuf")
                if veng:
                    nc.vector.tensor_copy(out=obuf, in_=ps_d)
                else:
                    nc.scalar.copy(out=obuf, in_=ps_d)
                deng = nc.sync if veng else nc.gpsimd
                deng.dma_start(out=out[b, 128 * j:128 * (j + 1), :], in_=obuf)

            def emit_dense(b, j):
                s = st[b]
                ps_d = psB.tile([128, D], f32, tag="ps512")
                for dc in range(NDC):
                    nc.tensor.matmul(
                        ps_d,
                        s["concat"][:, dc, 128 * j:128 * (j + 1)],
                        dw_sb[:, dc, :],
                        start=(dc == 0),
                        stop=(dc == NDC - 1) and not use_bias)
                finish_dense(b, j, ps_d, False)

            # ---- emission schedule ----
            # Warm-up: the PE would idle ~7us waiting for the first loads
            # (fixed ~7us framework preamble before any DMA issue) and then
            # run another ~3.4us at half clock (HAM). Burn that dead time on
            # dummy matmuls over a memset scratch tile so the clock-gate is
            # already released when the real work arrives (~14us in). N=128
            # keeps the granularity fine so the real matmuls aren't blocked
            # long past data arrival.
            with tc.tile_pool(name="psW", bufs=1, space="PSUM") as psW:
                wps = psW.tile([128, 128], f32, tag="warmps")
                for _ in range(40):
                    nc.tensor.matmul(wps, dummy[:, 0:128], dummy[:, 128:256],
                                     start=True, stop=True)

            # Phase A: batch-0 k (dc-outer groups first — they run during
            # the DMA ramp — then the rest j-outer), then v, then q^T,
            # projections (kt + batch-1 qT fillers interleaved so the PE
            # isn't gated on kt's DVE copies).
            with tc.tile_pool(name="psS", bufs=2, space="PSUM") as psS:
                st[0]["knat"] = bigp.tile([128, NJ, D], bf16, tag="knat",
                                          name="knat_0")
                st[0]["vnat"] = bigp.tile([128, NJ, D], bf16, tag="vnat",
                                          name="vnat_0")
                emit_k_group(0, 0)
                emit_k_group(0, 1)
                for j in range(8, NJ):
                    emit_k(0, j, which=("knat",))
                for j in range(NJ):
                    emit_k(0, j, which=("vnat",))
                for pr in range(NPAIR):
                    for n in range(SCH):
                        emit_qt(0, pr, n)
                for pr in range(NPAIR):
                    emit_proj(0, pr, psS)
                    emit_qt(1, 0, pr)  # filler (batch-1 qT, pair 0)
                    emit_kt(0, pr, psS)

                # Phase B: batch-0 attention (software-pipelined) interleaved
                # with batch-1 kv+qT fillers (2/round early, 1/round late so
                # the last pipeline rounds aren't bare).
                fillers = [(emit_kv, (1, j)) for j in range(NJ)] + \
                          [(emit_qt, (1, pr, n)) for pr in range(1, NPAIR)
                           for n in range(SCH)]
                fi = 0
                prev = None
                for k, (pr, n) in enumerate((pr, n) for pr in range(NPAIR)
                                            for n in range(SCH)):
                    emit_att_a(0, pr, n)
                    for _ in range(2 if k < 12 else 1):
                        if fi < len(fillers):
                            f, a = fillers[fi]; f(*a); fi += 1
                    if prev is not None:
                        emit_att_b(0, *prev)
                    prev = (pr, n)
                while fi < len(fillers):
                    f, a = fillers[fi]; f(*a); fi += 1
                emit_att_b(0, *prev)

                # Phase C: batch-1 projections + tap tables, with batch-0
                # dense interleaved to keep the PE fed.
                dq = [(0, j) for j in range(NJ)]
                for pr in range(NPAIR):
                    emit_proj(1, pr, psS)
                    emit_dense(*dq.pop(0))
                    emit_kt(1, pr, psS)
                    emit_dense(*dq.pop(0))

            # Phase D: batch-1 attention (n-outer, software-pipelined)
            # interleaved with remaining dense from ready-queues. The last
            # s-chunk's dense runs as per-pair PSUM partials (lagging the
            # normalize by 2 rounds) so the kernel tail stays short.
            with tc.tile_pool(name="psD", bufs=4, space="PSUM") as psD:
                ps_acc = {}
                JL = list(range(SCH * (SCH - 1), NJ))  # last-chunk j tiles

                def emit_dense_partials(pr):
                    for j in JL:
                        if pr == 0:
                            ps_acc[j] = psD.tile([128, D], f32, tag="psd",
                                                 name=f"psd_{j}")
                        nc.tensor.matmul(
                            ps_acc[j],
                            st[1]["concat"][:, pr, 128 * j:128 * (j + 1)],
                            dw_sb[:, pr, :],
                            start=(pr == 0),
                            stop=(pr == NPAIR - 1) and not use_bias)
                    if pr == NPAIR - 1:
                        for i, j in enumerate(JL):
                            finish_dense(1, j, ps_acc[j], veng=(i % 2 == 0))

                prev = None
                pend_partial = []
                for n in range(SCH):
                    for pr in range(NPAIR):
                        emit_att_a(1, pr, n)
                        if dq:
                            emit_dense(*dq.pop(0))
                        if pend_partial:
                            emit_dense_partials(pend_partial.pop(0))
                        if prev is not None:
                            ppr, pn = prev
                            emit_att_b(1, ppr, pn)
                            if pn == SCH - 1:
                                pend_partial.append(ppr)
                            elif ppr == NPAIR - 1:
                                dq.extend((1, jj) for jj in
                                          range(SCH * pn, SCH * (pn + 1)))
                        if dq:
                            emit_dense(*dq.pop(0))
                        prev = (pr, n)
                emit_att_b(1, *prev)
                pend_partial.append(prev[0])
                while dq:
                    emit_dense(*dq.pop(0))
                while pend_partial:
                    emit_dense_partials(pend_partial.pop(0))

    nc.finalize()
    return nc


def _prep_inputs(x, mask, wq, wk, wv, EW, FW, conv_w1, conv_w3, conv_w5, conv_b,
                 dense_w, dense_b, cluster_table):
    """Host-side restructuring -> per-core input maps."""
    bf = ml_dtypes.bfloat16
    x = np.ascontiguousarray(np.asarray(x, np.float32))
    mask = np.asarray(mask)
    counts = np.clip(mask.astype(np.int64).sum(1), 1, S)
    pos = np.asarray(cluster_table)[counts - 1]          # [B, P, C]
    if not (pos == pos[0]).all():
        raise NotImplementedError("per-batch cluster tables not supported")
    p0 = pos[0]                                          # [P, C]

    scale = 1.0 / np.sqrt(np.float32(DEPTH))
    s_idx = p0.ravel()
    c_idx = np.repeat(np.arange(P), C)

    def build_table(W, sc):
        A = np.zeros((H, S + 1, P), np.float32)
        np.add.at(A, (np.arange(H)[:, None], s_idx[None, :], c_idx[None, :]),
                  np.asarray(W, np.float32).reshape(H, P * C) * sc)
        return np.ascontiguousarray(A[:, :S, :])

    AE = build_table(EW, scale)
    AF = build_table(FW, 1.0)
    # pack adjacent heads side by side: [NPAIR, S, 128], then lay out for
    # contiguous DMA into [128, NJ, 128] SBUF tiles (partition = s % 128).
    def pack(A):
        A = A.reshape(NPAIR, 2, S, P).transpose(0, 2, 1, 3).reshape(NPAIR, S, 128)
        A = A.reshape(NPAIR, NJ, 128, 128).transpose(0, 2, 1, 3)
        return np.ascontiguousarray(A.reshape(NPAIR, 128, NJ * 128))
    AE = pack(AE)
    AF = pack(AF)

    # conv -> 5 tap matrices
    wp = np.arange(P)[:, None]
    jj = np.arange(P)[None, :]
    ii = wp - jj + 31
    valid = (ii >= 0) & (ii < P)
    ii = np.clip(ii, 0, P - 1)
    M = {t: np.zeros((P, P), np.float32) for t in range(-2, 3)}
    for cw, hk in ((conv_w1, 1), (conv_w3, 3), (conv_w5, 5)):
        cw = np.asarray(cw, np.float32)
        pad = (hk - 1) // 2
        for dy in range(hk):
            filt = cw[dy, :, 0, 0]
            M[dy - pad] += np.where(valid, filt[ii], 0.0) / 3.0
    BDM = np.zeros((5, 128, 128), np.float32)
    for ti in range(5):
        BDM[ti, :64, :64] = M[ti - 2]
        BDM[ti, 64:, 64:] = M[ti - 2]
    BDM = np.ascontiguousarray(BDM.transpose(1, 0, 2).reshape(128, 5 * 128))
    bbar = float(np.asarray(conv_b, np.float32).mean())
    if abs(bbar) > 1e-30:
        raise NotImplementedError("nonzero conv bias not folded")

    ones_bd = np.zeros((128, 128), np.float32)
    ones_bd[:64, :64] = 1.0
    ones_bd[64:, 64:] = 1.0

    def wlay(w):
        # [D, D] -> [128, NDC*D] so SBUF tile [128, NDC, D] loads contiguous
        w = np.asarray(w, np.float32).reshape(NDC, 128, D).transpose(1, 0, 2)
        return np.ascontiguousarray(w.reshape(128, NDC * D))

    db = np.asarray(dense_b, np.float32)
    use_bias = bool(np.any(db != 0.0))

    # shard + transpose x
    xsh = x.reshape(NCORES, BLOC, S, D)
    in_maps = []
    shared = dict(
        wq=wlay(wq).astype(bf),
        wk=wlay(wk).astype(bf),
        wv=wlay(wv).astype(bf),
        dw=wlay(dense_w).astype(bf),
        dbb=db.reshape(1, -1).astype(bf),
        ae=AE.astype(bf), af=AF.astype(bf), bdm=BDM.astype(bf),
        onesbd=ones_bd.astype(bf),
    )
    for c in range(NCORES):
        m = dict(shared)
        m["xT"] = np.ascontiguousarray(xsh[c].transpose(0, 2, 1)).astype(bf)
        in_maps.append(m)
    return in_maps, use_bias


def _run(prep, trace=False, tmpdir=None):
    from concourse.bass_utils import run_bass_kernel_spmd
    in_maps, use_bias = prep
    key = ("nc", use_bias)
    if key not in _CACHE:
        _CACHE[key] = _build_nc(use_bias)
    kw = {}
    if trace:
        _install_ntff_hook()
        kw = dict(trace=True, tmpdir=tmpdir)
    return run_bass_kernel_spmd(_CACHE[key], in_maps,
                                core_ids=list(range(NCORES)), **kw)


def _install_ntff_hook():
    import types, importlib.util as ilu
    if "antenv.axon_hooks" in sys.modules:
        return
    spec = ilu.spec_from_file_location(
        "trn_boot_mod", "/root/.axon_site/trn_agent_boot/trn_boot.py")
    tb = ilu.module_from_spec(spec)
    spec.loader.exec_module(tb)
    hook = tb._ntff_profile_via_ctypes("/opt/axon/libaxon_pjrt.so")
    mod = types.ModuleType("antenv.axon_hooks")
    mod.get_axon_ntff_profile_hook = lambda: hook
    import antenv  # noqa: F401
    sys.modules["antenv.axon_hooks"] = mod


def kernel(**inputs) -> np.ndarray:
    prep = _prep_inputs(**inputs)
    r = _run(prep)
    return np.concatenate([r.results[c]["out"] for c in range(NCORES)], axis=0)
